# revision 27
# baseline (speedup 1.0000x reference)
"""FFF (fast feedforward / MoE-routing binary tree) forward pass on 8 Trainium2 NeuronCores.

Strategy (data-parallel over the 16384-token batch, 2048 tokens/core):
  - Levels 0..9 (1023 nodes) are computed DENSE: logits via PE fp32 matmul,
    tree walk via one-hot map maintenance on DVE (bf16), masked acts @ w_out.T
    via PE in bf16.
  - Levels 10..11 (3072 nodes) are computed SPARSE with ONE gather per token:
    after the walk reaches level 10, a single indirect DMA pulls a per-node
    blob holding [w_in(n10) fp32 | w_out(n10), w_in(children), w_out(children)
    bf16].  The level-10 logit is a fp32 DVE dot (decision-grade); both leaf
    children's logits are bf16 dots and the wrong child is zeroed via its
    coefficient, so no data-dependent select is needed.  Contributions enter
    the same output PSUM banks via diagonal bf16 matmuls.
  - Node numbering is relabeled (within-level bit-reversal, level-d block at
    free-dim offset 2^d) so every walk update is a pair of CONTIGUOUS
    tensor_tensor ops (bf16 2x mode) and the level-10 index is recovered with
    a single iota reduction instead of per-level pick extraction.
"""

import numpy as np

P = 128
D = 1024
KC = 8                 # 1024 / 128 contraction chunks
DEPTH = 11
DN = 1024              # dense slots: levels 0..9 (1023 nodes) + 1 pad at slot 0
N_CORES = 8
TOK = 2048             # tokens per core
NT = TOK // P          # 16 token tiles per core
BW = 5 * D             # blobB row width (bf16 words)


def build_nc():
    from concourse import bacc, bass, mybir, tile
    from concourse.masks import make_identity

    dt = mybir.dt
    AFT = mybir.ActivationFunctionType
    ALU = mybir.AluOpType

    nc = bacc.Bacc("TRN2", target_bir_lowering=False, debug=False)

    x_d = nc.dram_tensor("x", [TOK, D], dt.float32, kind="ExternalInput")
    xT_d = nc.dram_tensor("xT", [NT, P, D], dt.float32, kind="ExternalInput")
    w_inT_d = nc.dram_tensor("w_inT_dn", [KC, P, DN], dt.float32, kind="ExternalInput")
    woT_d = nc.dram_tensor("woT_dn", [P, KC * D], dt.bfloat16, kind="ExternalInput")
    blob_d = nc.dram_tensor("blob", [DN, D + BW // 2], dt.float32, kind="ExternalInput")
    out_d = nc.dram_tensor("out", [TOK, D], dt.bfloat16, kind="ExternalOutput")

    from contextlib import ExitStack

    with tile.TileContext(nc) as tc, ExitStack() as es:
        pool_specs = [
            ("const", 1, None), ("xT", 2, None), ("xn", 3, None),
            ("xnh", 3, None), ("nmap", 2, None), ("dec", 2, None),
            ("acts", 2, None), ("msk", 2, None), ("mskT", 2, None),
            ("gw", 3, None), ("dscr", 2, None),
            ("prods", 2, None), ("osb", 2, None), ("tiny", 4, None),
            ("lps", 2, "PSUM"), ("tps", 2, "PSUM"), ("ops", 1, "PSUM"),
        ]
        pools = {}
        for pname, bufs, spc in pool_specs:
            kw = {"name": pname, "bufs": bufs}
            if spc is not None:
                kw["space"] = spc
            pools[pname] = es.enter_context(tc.tile_pool(**kw))
        (cpool, xT_pool, xn_pool, xnh_pool, map_pool, dec_pool, acts_pool,
         msk_pool, mskT_pool, gw_pool, dscr_pool, prods_pool,
         osb_pool, tiny_pool, lps_pool, tps_pool, ops_pool) = (
            pools[n] for n, _, _ in pool_specs)
        if True:
            identb = cpool.tile([P, P], dt.bfloat16)
            make_identity(nc, identb[:])
            iotaf = cpool.tile([P, 512], dt.float32)
            nc.gpsimd.iota(
                iotaf[:], pattern=[[1, 512]], base=0, channel_multiplier=0,
                allow_small_or_imprecise_dtypes=True,
            )
            w_inT_sb = cpool.tile([P, KC * DN], dt.float32)
            for k in range(KC):
                nc.sync.dma_start(
                    out=w_inT_sb[:, k * DN:(k + 1) * DN],
                    in_=w_inT_d[k],
                )
            woT_sb = cpool.tile([P, KC * D], dt.bfloat16)
            nc.sync.dma_start(out=woT_sb[:], in_=woT_d[:])

            def stage_a(t):
                st = {}
                xT = xT_pool.tile([P, D], dt.float32)
                nc.sync.dma_start(out=xT[:], in_=xT_d[t])
                xn = xn_pool.tile([P, D], dt.float32)
                nc.sync.dma_start(out=xn[:], in_=x_d[t * P:(t + 1) * P, :])
                xnh = xnh_pool.tile([P, D], dt.bfloat16)
                nc.scalar.copy(out=xnh[:], in_=xn[:])
                st["xn"], st["xnh"] = xn, xnh

                # dense logits for levels 0..9: (128 tokens, 1024 slots)
                lps = lps_pool.tile([P, DN], dt.float32, space="PSUM")
                acts = acts_pool.tile([P, DN], dt.bfloat16)
                for k in range(KC):
                    for h in range(2):
                        nc.tensor.matmul(
                            out=lps[:, h * 512:(h + 1) * 512],
                            lhsT=xT[:, k * P:(k + 1) * P],
                            rhs=w_inT_sb[:, k * DN + h * 512: k * DN + (h + 1) * 512],
                            start=(k == 0),
                            stop=(k == KC - 1),
                        )
                dec = dec_pool.tile([P, 512], dt.bfloat16, tag="dec_lo")
                nc.vector.tensor_scalar(
                    out=dec[:], in0=lps[:, 0:512], scalar1=0.0, scalar2=None,
                    op0=ALU.is_gt,
                )
                dec_hi = dec_pool.tile([P, 512], dt.float32, tag="dec_hi")
                nc.vector.tensor_scalar(
                    out=dec_hi[:], in0=lps[:, 512:1024], scalar1=0.0, scalar2=None,
                    op0=ALU.is_gt,
                )
                nc.scalar.activation(out=acts[:], in_=lps[:], func=AFT.Gelu)

                # walk: one-hot map, level-d block at [2^d, 2^{d+1})
                mp = map_pool.tile([P, DN], dt.bfloat16)
                nc.vector.memset(mp[:, 0:1], 0.0)
                nc.vector.memset(mp[:, 1:2], 1.0)
                nc.vector.tensor_copy(out=mp[:, 3:4], in_=dec[:, 1:2])
                nc.vector.tensor_scalar(
                    out=mp[:, 2:3], in0=dec[:, 1:2],
                    scalar1=-1.0, scalar2=1.0, op0=ALU.mult, op1=ALU.add,
                )
                for d in range(1, 9):
                    w = 2 ** d
                    nc.vector.tensor_tensor(
                        out=mp[:, 3 * w:4 * w], in0=mp[:, w:2 * w],
                        in1=dec[:, w:2 * w], op=ALU.mult,
                    )
                    nc.vector.tensor_tensor(
                        out=mp[:, 2 * w:3 * w], in0=mp[:, w:2 * w],
                        in1=mp[:, 3 * w:4 * w], op=ALU.subtract,
                    )

                # level-10 index: i10 = sum(map9 * (iota + 512*dec9))
                uvec = tiny_pool.tile([P, 512], dt.float32, tag="uvec")
                nc.vector.scalar_tensor_tensor(
                    out=uvec[:], in0=dec_hi[:], scalar=512.0,
                    in1=iotaf[:], op0=ALU.mult, op1=ALU.add,
                )
                m9f = tiny_pool.tile([P, 512], dt.float32, tag="m9f")
                nc.vector.tensor_copy(out=m9f[:], in_=mp[:, 512:1024])
                t1 = tiny_pool.tile([P, 512], dt.float32, tag="t1")
                i10f = tiny_pool.tile([P, 1], dt.float32, tag="i10f")
                nc.vector.scalar_tensor_tensor(
                    out=t1[:], in0=uvec[:], scalar=1.0, in1=m9f[:],
                    op0=ALU.mult, op1=ALU.mult, accum_out=i10f[:],
                )
                idx = tiny_pool.tile([P, 1], dt.int32, tag="idx")
                nc.vector.tensor_copy(out=idx[:], in_=i10f[:])

                # one merged gather for levels 10+11
                gw = gw_pool.tile([P, D + BW // 2], dt.float32)
                nc.gpsimd.indirect_dma_start(
                    out=gw[:], out_offset=None, in_=blob_d[:],
                    in_offset=bass.IndirectOffsetOnAxis(ap=idx[:], axis=0),
                )
                st["gw"] = gw

                # mask + transpose (bf16)
                msk = msk_pool.tile([P, DN], dt.bfloat16)
                nc.gpsimd.tensor_tensor(
                    out=msk[:], in0=acts[:], in1=mp[:], op=ALU.mult
                )
                tps = tps_pool.tile([P, DN], dt.bfloat16, space="PSUM")
                for c in range(KC):
                    nc.tensor.transpose(
                        out=tps[:, c * P:(c + 1) * P],
                        in_=msk[:, c * P:(c + 1) * P],
                        identity=identb[:],
                    )
                mskT = mskT_pool.tile([P, DN], dt.bfloat16)
                nc.scalar.copy(out=mskT[:], in_=tps[:])
                st["mskT"] = mskT
                return st

            def stage_b(t, st):
                xn, xnh, gw, mskT = st["xn"], st["xnh"], st["gw"], st["mskT"]
                # level-10 fp32 dot -> decision + coef
                dscr = dscr_pool.tile([P, D], dt.float32)
                l10 = tiny_pool.tile([P, 1], dt.float32, tag="l10")
                nc.vector.scalar_tensor_tensor(
                    out=dscr[:], in0=xn[:], scalar=1.0, in1=gw[:, 0:D],
                    op0=ALU.mult, op1=ALU.mult, accum_out=l10[:],
                )
                dec10 = tiny_pool.tile([P, 1], dt.float32, tag="dec10")
                nc.vector.tensor_scalar(
                    out=dec10[:], in0=l10[:], scalar1=0.0, scalar2=None,
                    op0=ALU.is_gt,
                )
                c10 = tiny_pool.tile([P, 1], dt.float32, tag="c10")
                nc.scalar.activation(out=c10[:], in_=l10[:], func=AFT.Gelu)

                # both leaf children's bf16 dots; zero the unchosen one
                prods = prods_pool.tile([P, 2 * D], dt.bfloat16)
                clr = tiny_pool.tile([P, 2], dt.float32, tag="clr")
                nc.vector.scalar_tensor_tensor(
                    out=prods[:, 0:D], in0=xnh[:], scalar=1.0,
                    in1=gw[:, D + D // 2: D + D].bitcast(dt.bfloat16),
                    op0=ALU.mult, op1=ALU.mult, accum_out=clr[:, 0:1],
                )
                nc.vector.scalar_tensor_tensor(
                    out=prods[:, D:2 * D], in0=xnh[:], scalar=1.0,
                    in1=gw[:, D + D: D + 3 * D // 2].bitcast(dt.bfloat16),
                    op0=ALU.mult, op1=ALU.mult, accum_out=clr[:, 1:2],
                )
                cLR = tiny_pool.tile([P, 2], dt.float32, tag="cLR")
                nc.scalar.activation(out=cLR[:], in_=clr[:], func=AFT.Gelu)
                cl = tiny_pool.tile([P, 1], dt.float32, tag="cl")
                nc.vector.tensor_scalar(
                    out=cl[:], in0=dec10[:],
                    scalar1=-1.0, scalar2=1.0, op0=ALU.mult, op1=ALU.add,
                )
                nc.vector.tensor_tensor(
                    out=cl[:], in0=cl[:], in1=cLR[:, 0:1], op=ALU.mult
                )
                cr = tiny_pool.tile([P, 1], dt.float32, tag="cr")
                nc.vector.tensor_tensor(
                    out=cr[:], in0=dec10[:], in1=cLR[:, 1:2], op=ALU.mult
                )

                # diagonal coef tiles for the sparse contributions
                dg10 = tiny_pool.tile([P, P], dt.bfloat16, tag="dg10")
                nc.vector.tensor_scalar(
                    out=dg10[:], in0=identb[:], scalar1=c10[:], scalar2=None,
                    op0=ALU.mult,
                )
                dgl = tiny_pool.tile([P, P], dt.bfloat16, tag="dgl")
                nc.vector.tensor_scalar(
                    out=dgl[:], in0=identb[:], scalar1=cl[:], scalar2=None,
                    op0=ALU.mult,
                )
                dgr = tiny_pool.tile([P, P], dt.bfloat16, tag="dgr")
                nc.vector.tensor_scalar(
                    out=dgr[:], in0=identb[:], scalar1=cr[:], scalar2=None,
                    op0=ALU.mult,
                )

                # output accumulation: dense 8 chunks + 3 sparse diags
                ops = ops_pool.tile([P, D], dt.float32, space="PSUM")
                for h in range(2):
                    o0 = h * 512
                    for c in range(KC):
                        nc.tensor.matmul(
                            out=ops[:, o0:o0 + 512],
                            lhsT=mskT[:, c * P:(c + 1) * P],
                            rhs=woT_sb[:, c * D + o0: c * D + o0 + 512],
                            start=(c == 0),
                            stop=False,
                            skip_group_check=True,
                        )
                    for dg, boff in ((dg10, 0), (dgl, 3 * D), (dgr, 4 * D)):
                        b0 = D + (boff + o0) // 2
                        nc.tensor.matmul(
                            out=ops[:, o0:o0 + 512],
                            lhsT=dg[:],
                            rhs=gw[:, b0: b0 + 256].bitcast(dt.bfloat16),
                            start=False,
                            stop=(boff == 4 * D),
                            skip_group_check=True,
                        )
                osb = osb_pool.tile([P, D], dt.bfloat16)
                nc.scalar.copy(out=osb[:], in_=ops[:])
                nc.sync.dma_start(out=out_d[t * P:(t + 1) * P, :], in_=osb[:])

            SKEW = 1
            states = {}
            for t in range(NT + SKEW):
                if t < NT:
                    states[t] = stage_a(t)
                if t >= SKEW:
                    stage_b(t - SKEW, states.pop(t - SKEW))

    nc.compile()
    return nc


def _bitrev(i, bits):
    r = 0
    for _ in range(bits):
        r = (r << 1) | (i & 1)
        i >>= 1
    return r


def _dense_perm():
    """perm[s] = heap node id stored at dense slot s (slot 0 unused)."""
    perm = np.zeros(DN, np.int64)
    for d in range(10):
        w = 2 ** d
        i = np.arange(w)
        rev = np.array([_bitrev(int(j), d) for j in i], np.int64)
        perm[w + i] = (w - 1) + rev
    return perm


def _leaf_perm():
    """lperm[i] = heap id of the level-10 node stored at blob row i."""
    i = np.arange(1024)
    rev = np.array([_bitrev(int(j), 10) for j in i], np.int64)
    return 1023 + rev


_DENSE_PERM = _dense_perm()
_LEAF_PERM = _leaf_perm()


def host_prep(x, w_in, w_out):
    """Build the per-core input maps (host-side transposes/tilings)."""
    import ml_dtypes

    bf16 = ml_dtypes.bfloat16
    x = np.ascontiguousarray(x, np.float32)
    w_in = np.ascontiguousarray(w_in, np.float32)
    w_out = np.ascontiguousarray(w_out, np.float32)

    # dense (levels 0..9) weights in shifted-relabeled order; slot 0 = zeros
    w_in_dn = np.zeros((DN, D), np.float32)
    w_in_dn[1:] = w_in[_DENSE_PERM[1:]]
    w_inT_dn = np.ascontiguousarray(
        w_in_dn.T.reshape(KC, P, DN)
    )  # [k,p,n] = w_in_dn[n, k*128+p]

    w_outT = np.ascontiguousarray(w_out.T)  # (n_nodes, D)
    woT_dn = np.zeros((DN, D), np.float32)
    woT_dn[1:] = w_outT[_DENSE_PERM[1:]]
    woT_dn = np.ascontiguousarray(
        woT_dn.reshape(KC, P, D).transpose(1, 0, 2).reshape(P, KC * D).astype(bf16)
    )  # [p, c*D+o] = w_out_dn[o, c*128+p] in bf16

    n10 = _LEAF_PERM
    lc = 2 * n10 + 1
    rc = 2 * n10 + 2
    blobB = np.concatenate(
        [w_outT[n10], w_in[lc], w_in[rc], w_outT[lc], w_outT[rc]], axis=1
    ).astype(bf16)  # (1024, 5D) bf16
    blob = np.ascontiguousarray(
        np.concatenate([w_in[n10], blobB.view(np.float32)], axis=1)
    )  # (1024, D + 5D/2) fp32 words

    in_maps = []
    for c in range(N_CORES):
        xs = x[c * TOK:(c + 1) * TOK]
        xT = np.ascontiguousarray(
            xs.reshape(NT, P, KC, P).transpose(0, 3, 2, 1).reshape(NT, P, D)
        )  # [t,p,k*128+j] = xs[t*128+j, k*128+p]
        in_maps.append(
            {
                "x": np.ascontiguousarray(xs),
                "xT": xT,
                "w_inT_dn": w_inT_dn,
                "woT_dn": woT_dn,
                "blob": blob,
            }
        )
    return in_maps


_NC_CACHE = {}


def kernel(x, w_in, w_out, force_depth=None, **_ignored):
    from concourse.bass_utils import run_bass_kernel_spmd

    if "nc" not in _NC_CACHE:
        _NC_CACHE["nc"] = build_nc()
    nc = _NC_CACHE["nc"]

    in_maps = host_prep(np.asarray(x), np.asarray(w_in), np.asarray(w_out))
    res = run_bass_kernel_spmd(nc, in_maps, core_ids=list(range(N_CORES)))
    out = np.concatenate(
        [np.asarray(res.results[c]["out"], dtype=np.float32) for c in range(N_CORES)],
        axis=0,
    )
    return out


if __name__ == "__main__":
    import reference

    inputs = reference.setup_inputs()
    expected = np.asarray(reference.reference(**inputs))
    actual = kernel(**{k: np.asarray(v) for k, v in inputs.items()})
    err = np.abs(actual - expected).max()
    print("absmax err:", err)
